# revision 33
# baseline (speedup 1.0000x reference)
"""Trainium2 Bass kernel for nn_Net_53644141527184.

Computation (per batch b):
  For each branch br in {x1, x3, x5}:
    picked[b, g, p] = x_br[b, idx[br, g, p], p]          (channel gather, p = 0..255)
    grid = picked.reshape(B, 128, 16, 16)
    crop[b, g, i, j] = grid[b, g, oh[g]+i, ow[g]+j]      (per-group 14x14 crop)
  feats = concat(crops, axis=1)                          -> [B, 384, 14, 14]
  out = einsum('bchw,oc->bohw', feats, W)                -> [B, 768, 14, 14]

Strategy: shard the 196 output positions q across the 8 cores (25 per core,
core 7 re-computing 4 of core 6's); every core handles ALL 128 batches and
ALL 768 output channels for its q-range.

x is relaid out host-side (data-independent transpose) to xg[br, p, c, b]
bf16, so the batch vector of one (channel, position) pick is one 256B
contiguous run in HBM.  The channel gather, the per-group crop, and the
transpose to matmul layout then all collapse into a single device-side
gpsimd.dma_gather per (branch, q-block): descriptor j = q_local*128 + g
fetches row p_local*512 + idx[br, g, p] of the core's 4-grid-row window
(p = 16*(oh[g]+qi) + (ow[g]+qj), all host-computed index arithmetic) and
lands it at partition g, row q_local of the feats tile [128 g, nq, 128 b].
Only the picked channels ever leave HBM (2.5 MB/core vs 12.6 MB for full x).

Conv: per (q-chunk of <=4, oc chunk of 128) one PSUM tile [128 o, 512] f32
accumulates 3 bf16 matmuls (one per branch tile; contraction is exactly
3 x 128, no padding).  PSUM drains alternate between the Activation and DVE
engines into per-(oc, output-block) staging tiles (bf16), each DMA'd out as
soon as its last chunk lands (runs >= 1KB).  Output returns bf16
[768, 25*128] per core and is upcast/reassembled host-side (same
quantization as the conv's bf16 inputs; rel err ~4e-3, well inside the
2e-2 gate).

Schedule notes (timeline-balanced; all paths end within ~0.4us):
  - gather q-blocks [4, 8, 8, 5]: block 0 small so the first branch tile
    (the PE's first operand) lands as early as the idx-load + descgen
    latency chain allows (~6.7us); the SWDGE descriptor-gen backbone on the
    Pool engine (994ns fixed + 0.34ns/desc per gather, serial) is what pins
    the later arrivals.
  - warm-up matmuls (NWARM, rotating over 4 PSUM banks so no warm-up ever
    blocks on a prior one) keep the tensor engine continuously busy from
    ~1.5us; a blocked/idle PE resets its clock-ramp p-state and a cold PE
    runs matmuls at half speed for 3us.
  - blocks 0-1 accumulate branch-OUTER (all oc chunks for branch 0, then
    branch 1, branch 2) so the PE starts on the first landed branch tile
    and absorbs the later branches' DMA latency; dep-free filler matmuls
    bridge the block-0 -> block-1 arrival gap.  Blocks 2-3 are branch-inner
    per (oc, chunk), oc-staggered, so drains and output DMAs spread across
    the stream instead of bunching at the end.
  - output blocks [(0,12), (12,8), (20,5)] x 6 oc chunks: 18 store DMAs,
    each fired as soon as its chunks drain; the tail chunk of block 3 is
    1 q so the final drain->store chain after the last matmul is short.
"""

import numpy as np
import ml_dtypes
from contextlib import ExitStack

import concourse.bacc as bacc
import concourse.bass as bass
import concourse.tile as tile
import concourse.mybir as mybir
from concourse import bass_utils

N_CORES = 8
B = 128       # batches (all on every core)
C = 512
G = 128       # groups per branch
BR = 3
OC = 768
NQ = 196      # output positions (14x14)
NQC = 25      # q positions per core
WROWS = 4     # grid rows in each core's x window
WP = 16 * WROWS                 # positions in window
Q0 = [25 * c for c in range(7)] + [NQ - NQC]   # per-core q-range start
QBLOCKS = [4, 8, 8, 5]          # gather q-blocks (sum = NQC); <= 8 q per
                                # block: the device-side SWDGE ring caps one
                                # dma_gather at 1024 descriptors.  Block 0 is
                                # small so the PE can start early.
# output blocks (q offset, q length): each (oc, block) DMA fires as soon as
# its chunks drain; all runs >= 1KB
OUTBLOCKS = [(0, 12), (12, 8), (20, 5)]
NIDX_COLS = sum(BR * (qb * 128) // 16 for qb in QBLOCKS)   # 600
NIDX_COLS0 = BR * (QBLOCKS[0] * 128) // 16                 # block-0 slice
NWARM = 13    # PE warm-up matmuls (see _build_program)
WROT = 4      # PSUM banks the warm-up rotates over

_CACHE = {}


def _build_program():
    nc = bacc.Bacc("TRN2", target_bir_lowering=False, debug=False,
                   num_devices=N_CORES, dynamic_dma_scratch_size=65536)

    f32 = mybir.dt.float32
    bf16 = mybir.dt.bfloat16

    xg_d = nc.dram_tensor("xg", [BR, WP * C, B], bf16, kind="ExternalInput")
    idxt_d = nc.dram_tensor("idxt", [128, NIDX_COLS], mybir.dt.int16,
                            kind="ExternalInput")
    wt_d = nc.dram_tensor("wt", [128, BR * OC], bf16, kind="ExternalInput")
    out_d = nc.dram_tensor("out", [OC, NQC * B], bf16, kind="ExternalOutput")

    with tile.TileContext(nc) as tc, ExitStack() as ctx:
        cpool = ctx.enter_context(tc.tile_pool(name="const", bufs=1))
        fpool = ctx.enter_context(tc.tile_pool(name="feats", bufs=1))
        opool = ctx.enter_context(tc.tile_pool(name="ostage", bufs=1))
        psump = ctx.enter_context(tc.tile_pool(name="ps", bufs=1, space="PSUM"))

        # split idx load: block 0's slice lands first so its gather descgen
        # (the head of the whole pipeline) starts as early as possible
        idxt0 = cpool.tile([128, NIDX_COLS0], mybir.dt.int16)
        nc.sync.dma_start(idxt0[:], idxt_d.ap()[:, :NIDX_COLS0])
        idxt1 = cpool.tile([128, NIDX_COLS - NIDX_COLS0], mybir.dt.int16)
        nc.sync.dma_start(idxt1[:], idxt_d.ap()[:, NIDX_COLS0:])
        wtb = cpool.tile([128, BR * OC], bf16)
        nc.sync.dma_start(wtb[:], wt_d.ap())

        # gathers: q-block major, branch minor, so the 3 branch tiles of a
        # block (needed together by its accumulation groups) land back-to-back
        feats = [[None] * BR for _ in QBLOCKS]
        col = 0
        for ib, nqb in enumerate(QBLOCKS):
            nidx = nqb * 128
            for br in range(BR):
                ft = fpool.tile([128, nqb * B], bf16, name=f"f{ib}_{br}")
                dst3 = ft[:].rearrange("p (r e) -> p r e", e=B)
                if ib == 0:
                    idsl = idxt0[:, col:col + nidx // 16]
                else:
                    idsl = idxt1[:, col - NIDX_COLS0:
                                 col - NIDX_COLS0 + nidx // 16]
                nc.gpsimd.dma_gather(dst3, xg_d.ap()[br], idsl, nidx, nidx, B)
                feats[ib][br] = ft
                col += nidx // 16

        def lhsT(br, oc):
            return wtb[:, br * OC + oc * 128: br * OC + (oc + 1) * 128]

        # output staging per (oc, output block)
        ost = [[opool.tile([128, ol * B], bf16, name=f"o{iob}_{oc}")
                for oc in range(6)] for iob, (oq, ol) in enumerate(OUTBLOCKS)]

        drain_tick = 0

        def drain(dst, src):
            nonlocal drain_tick
            if drain_tick % 2 == 0:
                nc.scalar.copy(dst, src)
            else:
                nc.vector.tensor_copy(dst, src)
            drain_tick += 1

        psn = 0

        def ps_tile():
            nonlocal psn
            t = psump.tile([128, 4 * B], f32, name=f"ps{psn % 8}")
            psn += 1
            return t

        # PE warm-up: throwaway matmuls keep the tensor engine continuously
        # busy (and thus ramping to full clock) from ~1.5us until just after
        # the first gathered tile lands.  Rotate across all 8 PSUM banks so
        # no warm-up matmul ever blocks on a prior one (a blocked PE resets
        # the clock-ramp), and the chain ends after the first real matmul's
        # operands are ready so the seam doesn't block either.
        wz = cpool.tile([128, 4 * B], bf16)
        nc.vector.memset(wz[:], 0.0)
        for i in range(NWARM):
            wp = psump.tile([128, 4 * B], f32, name=f"ps{i % WROT}")
            nc.tensor.matmul(wp[:], wz[:, :128], wz[:], start=True, stop=True)

        def mm_group_br_outer(ib, chunks, dst_ost, dst_qoff):
            """start accumulating each chunk as soon as branch 0 lands"""
            pcs = [[ps_tile() for _ in range(6)] for _ in chunks]
            for br in range(BR):
                for ic, (coff, csz) in enumerate(chunks):
                    for oc in range(6):
                        nc.tensor.matmul(
                            pcs[ic][oc][:, :csz * B], lhsT(br, oc),
                            feats[ib][br][:, coff * B:(coff + csz) * B],
                            start=(br == 0), stop=(br == BR - 1))
            for ic, (coff, csz) in enumerate(chunks):
                for oc in range(6):
                    drain(dst_ost[oc][:, (dst_qoff + coff) * B:
                                      (dst_qoff + coff + csz) * B],
                          pcs[ic][oc][:, :csz * B])

        def mm_group_br_inner(ib, chunks, dst_ost, dst_qoff, oc,
                              split_drain=False):
            for (coff, csz) in chunks:
                pc = ps_tile()
                for br in range(BR):
                    nc.tensor.matmul(
                        pc[:, :csz * B], lhsT(br, oc),
                        feats[ib][br][:, coff * B:(coff + csz) * B],
                        start=(br == 0), stop=(br == BR - 1))
                dst = dst_ost[oc][:, (dst_qoff + coff) * B:
                                  (dst_qoff + coff + csz) * B]
                if split_drain and csz > 1:
                    h = csz // 2 * B
                    nc.scalar.copy(dst[:, :h], pc[:, :h])
                    nc.vector.tensor_copy(dst[:, h:csz * B], pc[:, h:csz * B])
                else:
                    drain(dst, pc[:, :csz * B])

        def store(iob, oc):
            oq, ol = OUTBLOCKS[iob]
            nc.sync.dma_start(
                out_d.ap()[oc * 128:(oc + 1) * 128, oq * B:(oq + ol) * B],
                ost[iob][oc][:])

        # early blocks branch-outer (PE starts on the first landed branch and
        # never stalls on the rest); late blocks branch-inner, oc-staggered,
        # each (oc, output block) DMA firing as soon as its chunks drain
        mm_group_br_outer(0, [(0, 4)], ost[0], 0)             # q 0..4
        # dep-free fillers bridge the PE from block 0's end to block 1's
        # first-branch arrival without stalling (a stalled PE may drop to a
        # lower clock p-state)
        for i in range(3):
            fp = psump.tile([128, 4 * B], f32, name=f"ps{6 + (i % 2)}")
            nc.tensor.matmul(fp[:], wz[:, :128], wz[:], start=True, stop=True)
        mm_group_br_outer(1, [(0, 4)], ost[0], 4)             # q 4..8
        for oc in range(6):
            mm_group_br_inner(1, [(4, 4)], ost[0], 4, oc)     # q 8..12
            store(0, oc)
        for oc in range(6):
            mm_group_br_inner(2, [(0, 4), (4, 4)], ost[1], 0, oc)  # q 12..20
            store(1, oc)
        for oc in range(6):
            mm_group_br_inner(3, [(0, 4), (4, 1)], ost[2], 0, oc)  # q 20..25
            store(2, oc)

    nc.compile()
    return nc


def _prep_idx(idx, offh, offw):
    """Per-core gather descriptor index arrays [128, NIDX_COLS] int16."""
    idx = np.asarray(idx).astype(np.int64)      # [3, 128, 256]
    oh = np.asarray(offh).astype(np.int64)      # [3, 128]
    ow = np.asarray(offw).astype(np.int64)
    g = np.arange(G)
    out = []
    for core in range(N_CORES):
        q0 = Q0[core]
        rlo = q0 // 14
        cols = np.empty((16, NIDX_COLS), np.int16)
        col = 0
        qoff = 0
        for nqb in QBLOCKS:
            q = q0 + qoff + np.arange(nqb)
            qi, qj = q // 14, q % 14
            for br in range(BR):
                # p[g, q] = grid position picked for (group, output position)
                p = 16 * (oh[br, :, None] + qi[None, :]) + \
                    (ow[br, :, None] + qj[None, :])
                val = (p - 16 * rlo) * C + idx[br, g[:, None], p]
                assert val.min() >= 0 and val.max() < WP * C
                flat = val.T.reshape(-1)        # j = q_local*128 + g
                ncols = len(flat) // 16
                cols[:, col:col + ncols] = flat.reshape(ncols, 16).T
                col += ncols
            qoff += nqb
        out.append(np.tile(cols, (8, 1)))
    return out


def kernel(x1, x3, x5, W, idx, offh, offw):
    x1 = np.asarray(x1, dtype=np.float32)
    x3 = np.asarray(x3, dtype=np.float32)
    x5 = np.asarray(x5, dtype=np.float32)
    W = np.asarray(W, dtype=np.float32)
    assert x1.shape == (B, C, 16, 16)

    if "nc" not in _CACHE:
        _CACHE["nc"] = _build_program()
    nc = _CACHE["nc"]

    # xg[br, p, c, b] = x_br[b, c, p]  (pure relayout)
    stack = np.stack([x1.reshape(B, C, 256),
                      x3.reshape(B, C, 256),
                      x5.reshape(B, C, 256)])          # [br, b, c, p]
    xg = np.ascontiguousarray(stack.transpose(0, 3, 2, 1)).astype(
        ml_dtypes.bfloat16)                            # [br, 256, 512, 128]

    # wt[g, br*768 + o] = W[o, br*128 + g]
    wt = np.ascontiguousarray(
        W.reshape(OC, BR, G).transpose(2, 1, 0).reshape(G, BR * OC)
    ).astype(ml_dtypes.bfloat16)

    idxts = _prep_idx(idx, offh, offw)

    in_maps = []
    for core in range(N_CORES):
        rlo = Q0[core] // 14
        win = np.ascontiguousarray(
            xg[:, rlo * 16:rlo * 16 + WP]).reshape(BR, WP * C, B)
        in_maps.append({"xg": win, "idxt": idxts[core], "wt": wt})

    res = bass_utils.run_bass_kernel_spmd(nc, in_maps, list(range(N_CORES)))

    out = np.empty((B, OC, NQ), np.float32)
    for core in range(N_CORES):
        o = np.asarray(res.results[core]["out"]).astype(np.float32)
        o = o.reshape(OC, NQC, B).transpose(2, 0, 1)   # [b, o, q_local]
        out[:, :, Q0[core]:Q0[core] + NQC] = o
    return out.reshape(B, OC, 14, 14)
